# revision 7
# baseline (speedup 1.0000x reference)
"""Trainium2 Bass kernel for sliding-window self-attention (nn_ExRestSelfAtten).

Reference computation (B=4, S=2048, D_in=2048, H=128, D_out=2, window=+-32):
    x = x + pos_enc(S, H)                       # pos_enc is [S, S], input-independent
    h = relu(x @ W1 + b1)                       # [B,S,H]
    q = h @ Wq ; k/v = neighbors(h) @ Wk/Wv     # neighbors: 65-wide sliding window
    scores = q . k / sqrt(H) ; w = softmax(scores)
    context = (w . v) / sqrt(H) ; out = context @ Wo + bo
    returns (out, w)

Sharding: 8 cores = batch(4) x sequence-half(2). Each core owns 1024 tokens and
receives a 32-token halo on each side (zero rows beyond sequence edges, which
reproduces the reference's zero-padding of h exactly since b1 == relu input 0).

Host-side work is layout only: shard slicing, a transpose of x to feed the
tensor engine's contraction layout, folding pos_enc@W1+b1 into an extra
contraction chunk (its weight block is the identity), and extracting the
65-wide diagonal band of the attention-weight blocks the device writes out.
"""

import math

import numpy as np

import concourse.bacc as bacc
import concourse.mybir as mybir
import concourse.tile as tile
from concourse.bass_utils import run_bass_kernel_spmd

ATTEN = 32
B, S, D_IN, H, D_OUT = 4, 2048, 2048, 128, 2
N_CORES = 8
T_OWN = 1024                 # tokens owned per core
T_EXT = T_OWN + 2 * ATTEN    # 1088 incl. halos
KCH = D_IN // 128 + 1        # 17 contraction chunks (last = pos-enc fold)
WWIN = 2 * ATTEN + 1         # 65
WIN = 192                    # per-128-token-block score window (128 + 2*32)
NBLK = T_OWN // 128          # 8
XT = 512                     # token-tile width for the W1 matmul
NEG_MASK = -1.0e5            # exp(NEG_MASK - max) == 0 in f32

F32 = mybir.dt.float32
F32R = mybir.dt.float32r

# float32r runs the big matmuls at 4x the fp32 rate; contraction noise is far
# below typical tolerances here, but keep a switch to fall back to exact fp32.
W1_USE_F32R = True
QKO_USE_F32R = False

_CACHE: dict = {}


def _r(ap):
    """bitcast an f32 AP to f32r for full-rate matmuls."""
    return ap.bitcast(F32R)


def build_nc():
    nc = bacc.Bacc("TRN2", target_bir_lowering=False, debug=False,
                   num_devices=N_CORES)

    xdt = F32R if W1_USE_F32R else F32
    xt = nc.dram_tensor("xt", [KCH * 128, T_EXT], xdt, kind="ExternalInput")
    w1 = nc.dram_tensor("w1", [KCH * 128, H], xdt, kind="ExternalInput")
    wq = nc.dram_tensor("wq", [H, H], F32, kind="ExternalInput")
    wk = nc.dram_tensor("wk", [H, H], F32, kind="ExternalInput")
    wv = nc.dram_tensor("wv", [H, H], F32, kind="ExternalInput")
    wo = nc.dram_tensor("wo", [H, D_OUT], F32, kind="ExternalInput")
    bo = nc.dram_tensor("bo", [D_OUT, 1], F32, kind="ExternalInput")
    msk = nc.dram_tensor("msk", [128, WIN], F32, kind="ExternalInput")

    o_out = nc.dram_tensor("o", [D_OUT, T_OWN], F32, kind="ExternalOutput")
    wm_out = nc.dram_tensor("wm", [128, NBLK, WIN], F32, kind="ExternalOutput")

    xt_r = xt.ap().rearrange("(k p) t -> p k t", p=128)
    w1_r = w1.ap().rearrange("(k p) d -> p k d", p=128)

    with tile.TileContext(nc) as tc:
        with (
            tc.tile_pool(name="xa", bufs=3) as xa_pool,
            tc.tile_pool(name="wts", bufs=1) as wts,
            tc.tile_pool(name="big", bufs=1) as big,
            tc.tile_pool(name="sb_blk", bufs=6) as sb_blk,
            tc.tile_pool(name="stats", bufs=8) as stats,
            tc.tile_pool(name="ps_pre", bufs=2, space="PSUM") as ps_pre,
            tc.tile_pool(name="ps_blk", bufs=6, space="PSUM") as ps_blk,
        ):
            # ---- replicated weights / constants ----
            w1_sb = wts.tile([128, KCH, H], xdt)
            nc.sync.dma_start(out=w1_sb[:, :, :], in_=w1_r[:, :, :])
            wq_sb = wts.tile([128, H], F32)
            nc.sync.dma_start(out=wq_sb[:, :], in_=wq[:, :])
            wk_sb = wts.tile([128, H], F32)
            nc.sync.dma_start(out=wk_sb[:, :], in_=wk[:, :])
            wv_sb = wts.tile([128, H], F32)
            nc.sync.dma_start(out=wv_sb[:, :], in_=wv[:, :])
            wo_sb = wts.tile([128, D_OUT], F32)
            nc.sync.dma_start(out=wo_sb[:, :], in_=wo[:, :])
            bo_sb = wts.tile([D_OUT, 1], F32)
            nc.sync.dma_start(out=bo_sb[:, :], in_=bo[:, :])
            msk_sb = wts.tile([128, WIN], F32)
            nc.sync.dma_start(out=msk_sb[:, :], in_=msk[:, :])
            ident = w1_sb[:, KCH - 1, :].bitcast(F32)  # identity chunk, f32 view

            hT = big.tile([128, T_EXT], F32)       # relu(x@W1+C)^T
            kkT = big.tile([128, T_EXT], F32)      # (h@Wk)^T
            vv = big.tile([128, NBLK + 1, H], F32)  # h@Wv, token-major chunks
            ctx_all = big.tile([128, T_OWN], F32)  # context^T
            aw_all = big.tile([128, NBLK, WIN], F32)  # softmaxed block weights
            out_sb = big.tile([D_OUT, T_OWN], F32)

            # ---- phase A: stream x^T, W1 matmul, relu -> hT; kk/vv chunks --
            tok_tiles = []
            t0 = 0
            while t0 < T_EXT:
                tok_tiles.append((t0, min(XT, T_EXT - t0)))
                t0 += XT

            for (t0, tw) in tok_tiles:
                xa = xa_pool.tile([128, KCH, XT], xdt, tag="xa")
                nc.sync.dma_start(out=xa[:, :, :tw], in_=xt_r[:, :, t0:t0 + tw])

                pre = ps_pre.tile([128, XT], F32, tag="pre")
                for k in range(KCH):
                    nc.tensor.matmul(pre[:, :tw], w1_sb[:, k, :], xa[:, k, :tw],
                                     start=(k == 0), stop=(k == KCH - 1))
                nc.scalar.activation(out=hT[:, t0:t0 + tw], in_=pre[:, :tw],
                                     func=mybir.ActivationFunctionType.Relu)

                # kk chunk for this token tile
                kk_ps = ps_pre.tile([128, XT], F32, tag="pre")
                lhsT, rhs = wk_sb[:, :], hT[:, t0:t0 + tw]
                if QKO_USE_F32R:
                    lhsT, rhs = _r(lhsT), _r(rhs)
                nc.tensor.matmul(kk_ps[:, :tw], lhsT, rhs)
                nc.vector.tensor_copy(out=kkT[:, t0:t0 + tw], in_=kk_ps[:, :tw])

                # vv chunks (token-major: h^T chunk is the stationary operand)
                for j in range(t0 // 128, (t0 + tw + 127) // 128):
                    cm = min(128, T_EXT - j * 128)
                    vv_ps = ps_blk.tile([128, 256], F32, tag="blk")
                    nc.tensor.matmul(vv_ps[:cm, :H],
                                     hT[:, j * 128:j * 128 + cm], wv_sb[:, :])
                    nc.vector.tensor_copy(out=vv[:cm, j, :], in_=vv_ps[:cm, :H])

            # ---- phase B: per-128-token-block attention ----
            for b in range(NBLK):
                own0 = ATTEN + b * 128   # ext offset of this block's tokens

                q_ps = ps_blk.tile([128, 256], F32, tag="blk")
                lhsT, rhs = wq_sb[:, :], hT[:, own0:own0 + 128]
                if QKO_USE_F32R:
                    lhsT, rhs = _r(lhsT), _r(rhs)
                nc.tensor.matmul(q_ps[:, :128], lhsT, rhs)
                q_sb = sb_blk.tile([128, 256], F32, tag="sblk")
                nc.scalar.copy(out=q_sb[:, :128], in_=q_ps[:, :128])

                sc_ps = ps_blk.tile([128, 256], F32, tag="blk")
                nc.tensor.matmul(sc_ps[:, :WIN], q_sb[:, :128],
                                 kkT[:, b * 128:b * 128 + WIN])

                sm = sb_blk.tile([128, 256], F32, tag="sblk")
                nc.vector.tensor_add(out=sm[:, :WIN], in0=sc_ps[:, :WIN],
                                     in1=msk_sb[:, :])
                nmax = stats.tile([128, 1], F32, tag="st")
                nc.vector.reduce_max(out=nmax[:, :], in_=sm[:, :WIN],
                                     axis=mybir.AxisListType.X, negate=True)
                sume = stats.tile([128, 1], F32, tag="st")
                nc.scalar.activation(out=aw_all[:, b, :], in_=sm[:, :WIN],
                                     func=mybir.ActivationFunctionType.Exp,
                                     bias=nmax[:, :], scale=1.0,
                                     accum_out=sume[:, :])
                rsum = stats.tile([128, 1], F32, tag="st")
                nc.vector.reciprocal(out=rsum[:, :], in_=sume[:, :])
                nc.vector.tensor_scalar_mul(out=aw_all[:, b, :],
                                            in0=aw_all[:, b, :],
                                            scalar1=rsum[:, :])

                # transpose the 192-wide weight block (2 PE transposes)
                wT_ps = ps_blk.tile([128, 256], F32, tag="blk")
                nc.tensor.transpose(wT_ps[:, :128], aw_all[:, b, 0:128], ident)
                nc.tensor.transpose(wT_ps[:64, 128:256], aw_all[:, b, 128:WIN],
                                    ident)
                wT = sb_blk.tile([128, 256], F32, tag="sblk")
                nc.vector.tensor_copy(out=wT[:, :128], in_=wT_ps[:, :128])
                nc.scalar.copy(out=wT[:64, 128:256], in_=wT_ps[:64, 128:256])

                # context^T block: contract over the 192 window tokens
                cx_ps = ps_blk.tile([128, 256], F32, tag="blk")
                nc.tensor.matmul(cx_ps[:, :128], vv[:, b, :], wT[:, :128],
                                 start=True, stop=False)
                nc.tensor.matmul(cx_ps[:, :128], vv[:64, b + 1, :],
                                 wT[:64, 128:256], start=False, stop=True)
                nc.vector.tensor_copy(out=ctx_all[:, b * 128:(b + 1) * 128],
                                      in_=cx_ps[:, :128])

            # ---- output projection ----
            for t0 in range(0, T_OWN, XT):
                op = ps_pre.tile([D_OUT, XT], F32, tag="pre")
                lhsT, rhs = wo_sb[:, :], ctx_all[:, t0:t0 + XT]
                if QKO_USE_F32R:
                    lhsT, rhs = _r(lhsT), _r(rhs)
                nc.tensor.matmul(op[:, :], lhsT, rhs)
                nc.scalar.activation(out=out_sb[:, t0:t0 + XT], in_=op[:, :],
                                     func=mybir.ActivationFunctionType.Identity,
                                     bias=bo_sb[:, :], scale=1.0)

            nc.sync.dma_start(out=o_out[:, :], in_=out_sb[:, :])
            nc.sync.dma_start(out=wm_out[:, :, :], in_=aw_all[:, :, :])

    nc.compile()
    return nc


def _pos_enc_np():
    """pos_enc faithful to the reference's f32 semantics (inf -> angle 0)."""
    pos = np.arange(S, dtype=np.float32)[:, None]
    j = np.arange(S)
    expo = (2.0 * (j // 2).astype(np.float32)) / np.float32(H)
    with np.errstate(over="ignore"):
        inv = np.power(np.float32(10000.0), expo, dtype=np.float32)
    angle = pos / inv[None, :]
    return np.where((j % 2 == 0)[None, :], np.sin(angle), np.cos(angle)).astype(
        np.float32)


def _prep_inputs(x, W1, b1, Wq, Wk, Wv, Wo, bo):
    x = np.asarray(x, dtype=np.float32)
    W1 = np.asarray(W1, dtype=np.float32)
    b1 = np.asarray(b1, dtype=np.float32)
    scale = np.float32(1.0 / math.sqrt(float(H)))

    P = _pos_enc_np()
    C = P @ W1 + b1[None, :]                     # [S, H] pos-enc fold

    w1_aug = np.concatenate(
        [W1, np.eye(128, dtype=np.float32)], axis=0)  # [2176, 128]

    r = np.arange(128)[:, None]
    c = np.arange(WIN)[None, :]
    msk = np.where((c >= r) & (c <= r + 2 * ATTEN), np.float32(0.0),
                   np.float32(NEG_MASK)).astype(np.float32)

    shared = {
        "w1": np.ascontiguousarray(w1_aug),
        "wq": np.ascontiguousarray(np.asarray(Wq, np.float32) * scale),
        "wk": np.ascontiguousarray(np.asarray(Wk, np.float32)),
        "wv": np.ascontiguousarray(np.asarray(Wv, np.float32)),
        "wo": np.ascontiguousarray(np.asarray(Wo, np.float32) * scale),
        "bo": np.ascontiguousarray(np.asarray(bo, np.float32).reshape(D_OUT, 1)),
        "msk": msk,
    }

    in_maps = []
    for core in range(N_CORES):
        bb, half = divmod(core, 2)
        own0 = half * T_OWN
        lo, hi = own0 - ATTEN, own0 + T_OWN + ATTEN
        vlo, vhi = max(lo, 0), min(hi, S)
        xt_aug = np.zeros((KCH * 128, T_EXT), dtype=np.float32)
        xt_aug[:D_IN, vlo - lo:vhi - lo] = x[bb, vlo:vhi, :].T
        xt_aug[D_IN:, vlo - lo:vhi - lo] = C[vlo:vhi, :].T
        in_maps.append({"xt": np.ascontiguousarray(xt_aug), **shared})
    return in_maps


def _assemble(results):
    out = np.empty((B, S, D_OUT), dtype=np.float32)
    aw = np.empty((B, S, WWIN), dtype=np.float32)
    r = np.arange(128)[:, None]
    j = np.arange(WWIN)[None, :]
    band_idx = r + 2 * ATTEN - j                     # [128, 65] in [0, 192)
    for core in range(N_CORES):
        bb, half = divmod(core, 2)
        own0 = half * T_OWN
        out[bb, own0:own0 + T_OWN, :] = results[core]["o"].T
        wm = results[core]["wm"].reshape(128, NBLK, WIN)
        for blk in range(NBLK):
            aw[bb, own0 + blk * 128: own0 + (blk + 1) * 128, :] = \
                np.take_along_axis(wm[:, blk, :], band_idx, axis=1)
    return out, aw


def kernel(x, W1, b1, Wq, Wk, Wv, Wo, bo):
    if "nc" not in _CACHE:
        _CACHE["nc"] = build_nc()
    nc = _CACHE["nc"]
    in_maps = _prep_inputs(x, W1, b1, Wq, Wk, Wv, Wo, bo)
    res = run_bass_kernel_spmd(nc, in_maps, core_ids=list(range(N_CORES)))
    return _assemble(res.results)
